# revision 26
# baseline (speedup 1.0000x reference)
"""FAVOR+ linear attention (Performer-style) Trainium2 Bass kernel.

Full inputs -> full output. Sharding: 8 cores = (batch b in 0..3) x (query
half in 0..1). Each core computes the KV summary over all 16384 key pixels
of its batch (duplicated within the pair; avoids collectives) and the
numerator/denominator for its 8192 query pixels.

Hardcoded problem shape: B=4, C=128, H=W=128, hid=128, heads=8, hd=dv=16.
"""

import numpy as np
from contextlib import ExitStack

import concourse.bass as bass
import concourse.tile as tile
from concourse import bacc, mybir
from concourse.bass_utils import run_bass_kernel_spmd

F32 = mybir.dt.float32
AF = mybir.ActivationFunctionType

N_HEADS = 8
HD = 16          # head dim for q/k and v
C = 128          # channels == hid
S = 128 * 128    # pixels per image
SQ = S // 2      # query pixels per core
SK = S           # key pixels per core (duplicated across the pair)
SC = 2048        # super-chunk (pixels)

_PROGRAM = None
DBG_KISC = 0


def _build_program(debug=False, loop_n=None):
    nc = bacc.Bacc()
    xs = nc.declare_dram_parameter("xs", [C, SQ], F32, isOutput=False)
    ys = nc.declare_dram_parameter("ys", [C, SK], F32, isOutput=False)
    cpack = nc.declare_dram_parameter("cpack", [128, 779], F32, isOutput=False)
    outp = nc.declare_dram_parameter("outp", [SQ, 128], F32, isOutput=True)
    outp_r = outp.rearrange("(n k p) c -> n p k c", k=SC // 128, p=128)
    if debug:
        dbg_t = nc.declare_dram_parameter("dbg_t", [128, SC], F32, isOutput=True)
        dbg_kf = nc.declare_dram_parameter("dbg_kf", [128, SC], F32, isOutput=True)
        dbg_v = nc.declare_dram_parameter("dbg_v", [128, SC], F32, isOutput=True)
        dbg_kvb = nc.declare_dram_parameter("dbg_kvb", [128, 136], F32, isOutput=True)
        dbg_qs = nc.declare_dram_parameter("dbg_qs", [128, SC], F32, isOutput=True)
        dbg_nm = nc.declare_dram_parameter("dbg_nm", [128, SC // 128, 8, 17], F32,
                                           isOutput=True)

    with tile.TileContext(nc) as tc, ExitStack() as ctx:
        singles = ctx.enter_context(tc.tile_pool(name="singles", bufs=1))
        inpool = ctx.enter_context(tc.tile_pool(name="inpool", bufs=3))
        tpool = ctx.enter_context(tc.tile_pool(name="tpool", bufs=2))
        fpool = ctx.enter_context(tc.tile_pool(name="fpool", bufs=2))
        vpool = ctx.enter_context(tc.tile_pool(name="vpool", bufs=2))
        npool = ctx.enter_context(tc.tile_pool(name="npool", bufs=2))
        opool = ctx.enter_context(tc.tile_pool(name="opool", bufs=2))
        pps = ctx.enter_context(tc.tile_pool(name="pps", bufs=2, space="PSUM"))
        ppkv = ctx.enter_context(tc.tile_pool(name="ppkv", bufs=1, space="PSUM"))

        consts = singles.tile([128, 779], F32)
        nc.sync.dma_start(out=consts, in_=cpack[:])
        wqt = consts[:, 0:128]
        wkt = consts[:, 128:256]
        bmat = consts[:, 256:384]
        fmat = consts[:, 384:512]
        ident = consts[:, 512:640]
        bq = consts[:, 640:641]
        bk = consts[:, 641:642]
        mask136 = consts[:, 642:778]
        ones1 = consts[:, 778:779]

        # Tiny ops so PE/DVE observe the consts DMA once up front; later
        # instructions then carry at most one new semaphore wait each
        # (walrus per-instruction sync-command slot limit).
        presync = pps.tile([128, 512], F32, tag="tps")
        nc.tensor.matmul(presync[:1, 0:1], lhsT=consts[:, 0:1],
                         rhs=consts[:, 0:1], start=True, stop=True)
        dve_sync = singles.tile([128, 1], F32)
        nc.vector.tensor_copy(dve_sync, consts[:, 0:1])

        def body():
            # ---- K phase: KV[hm, hv] (+ ksum col) accumulated in PSUM ----
            kvps = ppkv.tile([128, 129], F32, tag="kv")
            nkc = SK // SC
            for isc in range(nkc):
                y_t = inpool.tile([128, SC], F32, tag="inbuf")
                nc.sync.dma_start(out=y_t, in_=ys[:, isc * SC:(isc + 1) * SC])
                t_t = tpool.tile([128, SC], F32, tag="t")
                t2_t = tpool.tile([128, SC], F32, tag="t2")
                for j in range(SC // 512):
                    blk = slice(j * 512, (j + 1) * 512)
                    tps = pps.tile([128, 512], F32, tag="tps")
                    nc.tensor.matmul(tps, lhsT=wkt, rhs=y_t[:, blk],
                                     start=True, stop=True)
                    nc.vector.tensor_scalar_add(t_t[:, blk], tps, bk)
                    nc.vector.tensor_mul(t2_t[:, blk], t_t[:, blk], t_t[:, blk])
                kf_t = fpool.tile([128, SC], F32, tag="feat")
                v_t = vpool.tile([128, SC], F32, tag="v")
                for j in range(SC // 512):
                    blk = slice(j * 512, (j + 1) * 512)
                    kfps = pps.tile([128, 512], F32, tag="kfps")
                    vps = pps.tile([128, 512], F32, tag="vps")
                    for c in range(4):
                        cc = j * 4 + c
                        sl = slice(c * 128, (c + 1) * 128)
                        ch = slice(cc * 128, (cc + 1) * 128)
                        nc.tensor.matmul(kfps[:, sl], lhsT=t_t[:, ch], rhs=bmat,
                                         start=True, stop=False)
                        nc.tensor.matmul(kfps[:, sl], lhsT=t2_t[:, ch], rhs=fmat,
                                         start=False, stop=True)
                        nc.tensor.matmul(vps[:, sl], lhsT=y_t[:, ch], rhs=ident,
                                         start=True, stop=True)
                    nc.scalar.activation(kf_t[:, blk], kfps, AF.Exp)
                    nc.vector.tensor_copy(v_t[:, blk], vps)
                for cc in range(SC // 128):
                    ch = slice(cc * 128, (cc + 1) * 128)
                    gfirst = (isc == 0) and (cc == 0)
                    last = (isc == nkc - 1) and (cc == SC // 128 - 1)
                    nc.tensor.matmul(kvps[:, 0:128], lhsT=kf_t[:, ch],
                                     rhs=v_t[:, ch], start=gfirst, stop=last,
                                     skip_group_check=True)
                    # start=False even on the first chunk: start=True clears
                    # the has_written bits of the WHOLE bank (would wipe the
                    # KV columns just written); unset bits still
                    # overwrite-init.
                    nc.tensor.matmul(kvps[:, 128:129], lhsT=kf_t[:, ch],
                                     rhs=ones1, start=False, stop=last,
                                     skip_group_check=True)
                if debug and isc == DBG_KISC:
                    nc.sync.dma_start(out=dbg_t[:], in_=t_t)
                    nc.sync.dma_start(out=dbg_kf[:], in_=kf_t)
                    nc.sync.dma_start(out=dbg_v[:], in_=v_t)

            # ---- KVB: block-diagonal [KV | ksum] for the num matmul ----
            kvsb = npool.tile([128, 129], F32, tag="kvsb")
            nc.vector.tensor_copy(kvsb, kvps)
            kvb_raw = npool.tile([128, 8, 17], F32, tag="kvbr")
            nc.vector.tensor_copy(kvb_raw[:, :, 0:16],
                                  kvsb[:, 0:128].rearrange("p (h j) -> p h j", h=8))
            nc.vector.tensor_copy(kvb_raw[:, :, 16:17],
                                  kvsb[:, 128:129, None].to_broadcast([128, 8, 1]))
            kvb3 = npool.tile([128, 8, 17], F32, tag="kvb3")
            nc.vector.tensor_mul(kvb3, kvb_raw,
                                 mask136.rearrange("p (h j) -> p h j", h=8))
            kvb = kvb3.rearrange("p h j -> p (h j)")
            if debug:
                nc.sync.dma_start(out=dbg_kvb[:], in_=kvb)

            # ---- Q phase ----
            for isc in range(SQ // SC):
                x_t = inpool.tile([128, SC], F32, tag="inbuf")
                nc.sync.dma_start(out=x_t, in_=xs[:, isc * SC:(isc + 1) * SC])
                t_t = tpool.tile([128, SC], F32, tag="t")
                t2_t = tpool.tile([128, SC], F32, tag="t2")
                for j in range(SC // 512):
                    blk = slice(j * 512, (j + 1) * 512)
                    tps = pps.tile([128, 512], F32, tag="tps")
                    nc.tensor.matmul(tps, lhsT=wqt, rhs=x_t[:, blk],
                                     start=True, stop=True)
                    nc.vector.tensor_scalar_add(t_t[:, blk], tps, bq)
                    nc.vector.tensor_mul(t2_t[:, blk], t_t[:, blk], t_t[:, blk])
                qs_t = fpool.tile([128, SC], F32, tag="feat")
                for j in range(SC // 512):
                    blk = slice(j * 512, (j + 1) * 512)
                    qps = pps.tile([128, 512], F32, tag="kfps")
                    nc.tensor.matmul(qps, lhsT=bmat, rhs=t_t[:, blk],
                                     start=True, stop=False)
                    nc.tensor.matmul(qps, lhsT=fmat, rhs=t2_t[:, blk],
                                     start=False, stop=True)
                    nc.scalar.activation(qs_t[:, blk], qps, AF.Exp)
                nm_t = npool.tile([128, SC // 128, 8, 17], F32, tag="nm")
                for cc in range(SC // 128):
                    ch = slice(cc * 128, (cc + 1) * 128)
                    nps = pps.tile([128, 136], F32, tag="vps")
                    nc.tensor.matmul(nps, lhsT=qs_t[:, ch], rhs=kvb,
                                     start=True, stop=True)
                    nc.scalar.activation(nm_t[:, cc],
                                         nps.rearrange("p (h j) -> p h j", h=8),
                                         AF.Copy)
                if debug and isc == 0:
                    nc.sync.dma_start(out=dbg_qs[:], in_=qs_t)
                    nc.sync.dma_start(out=dbg_nm[:], in_=nm_t)
                rden = npool.tile([128, SC // 128, 8], F32, tag="rden")
                nc.vector.reciprocal(rden, nm_t[:, :, :, 16])
                out_t = opool.tile([128, SC // 128, 128], F32, tag="outb")
                for h in range(N_HEADS):
                    nc.vector.tensor_mul(
                        out_t[:, :, 16 * h:16 * h + 16],
                        nm_t[:, :, h, 0:16],
                        rden[:, :, h, None].to_broadcast([128, SC // 128, 16]),
                    )
                nc.sync.dma_start(out=outp_r[isc], in_=out_t)

        if loop_n is None:
            body()
        else:
            with tc.For_i(0, loop_n, 1):
                body()

    nc.compile()
    return nc


def _get_program():
    global _PROGRAM
    if _PROGRAM is None:
        _PROGRAM = _build_program()
    return _PROGRAM


def _host_consts(rfs, Wq, bq, Wk, bk):
    scale = HD ** -0.25  # == 0.5 exactly
    cpack = np.zeros((128, 779), dtype=np.float32)
    cpack[:, 0:128] = (scale * Wq).T
    cpack[:, 128:256] = (scale * Wk).T
    bmat = np.zeros((128, 128), dtype=np.float32)
    fmat = np.zeros((128, 128), dtype=np.float32)
    for h in range(N_HEADS):
        bmat[16 * h:16 * h + 16, 16 * h:16 * h + 16] = rfs[h]
        fmat[16 * h:16 * h + 16, 16 * h:16 * h + 16] = -0.5
    cpack[:, 256:384] = bmat
    cpack[:, 384:512] = fmat
    cpack[:, 512:640] = np.eye(128, dtype=np.float32)
    cpack[:, 640] = scale * bq
    cpack[:, 641] = scale * bk
    mask = np.zeros((128, 136), dtype=np.float32)
    for h in range(N_HEADS):
        mask[16 * h:16 * h + 16, 17 * h:17 * h + 17] = 1.0
    cpack[:, 642:778] = mask
    cpack[:, 778] = 1.0
    return cpack


def make_in_maps(inputs):
    x = np.ascontiguousarray(np.asarray(inputs["x"], dtype=np.float32))
    y = np.ascontiguousarray(np.asarray(inputs["y"], dtype=np.float32))
    cpack = _host_consts(np.asarray(inputs["rfs"], dtype=np.float32),
                         np.asarray(inputs["Wq"], dtype=np.float32),
                         np.asarray(inputs["bq"], dtype=np.float32),
                         np.asarray(inputs["Wk"], dtype=np.float32),
                         np.asarray(inputs["bk"], dtype=np.float32))
    B = x.shape[0]
    xr = x.reshape(B, C, S)
    yr = y.reshape(B, C, S)
    in_maps = []
    for core in range(8):
        b, half = core // 2, core % 2
        s0 = half * SQ
        in_maps.append({
            "xs": np.ascontiguousarray(xr[b][:, s0:s0 + SQ]),
            "ys": np.ascontiguousarray(yr[b]),
            "cpack": cpack,
        })
    return in_maps


def run(inputs, trace=False, **kwargs):
    in_maps = make_in_maps(inputs)
    nc = _get_program()
    res = run_bass_kernel_spmd(nc, in_maps, list(range(8)), trace=trace, **kwargs)
    B = np.asarray(inputs["x"]).shape[0]
    out = np.empty((B, S, 128), dtype=np.float32)
    for core in range(8):
        b, half = core // 2, core % 2
        s0 = half * SQ
        out[b, s0:s0 + SQ, :] = res.results[core]["outp"]
    return out.reshape(np.asarray(inputs["x"]).shape), res


def kernel(**inputs):
    out, _ = run(inputs, trace=False)
    return out


# revision 36
# speedup vs baseline: 2.6864x; 2.6864x over previous
"""FAVOR+ linear attention (Performer-style) Trainium2 Bass kernel.

Full inputs -> full output. Sharding: 8 cores = (batch b in 0..3) x (query
half in 0..1). Each core computes the KV summary over all 16384 key pixels
of its batch (duplicated within the pair; avoids collectives) and the
numerator/denominator for its 8192 query pixels.

Hardcoded problem shape: B=4, C=128, H=W=128, hid=128, heads=8, hd=dv=16.
"""

import numpy as np
from contextlib import ExitStack

import concourse.bass as bass
import concourse.tile as tile
from concourse import bacc, mybir
from concourse.bass_utils import run_bass_kernel_spmd

F32 = mybir.dt.float32
F32R = mybir.dt.float32r
BF16 = mybir.dt.bfloat16
AF = mybir.ActivationFunctionType


def _r(ap):
    return ap.bitcast(F32R)

N_HEADS = 8
HD = 16          # head dim for q/k and v
C = 128          # channels == hid
S = 128 * 128    # pixels per image
SQ = S // 2      # query pixels per core
SK = S           # key pixels per core (duplicated across the pair)
SC = 2048        # super-chunk (pixels)

_PROGRAM = None
DBG_KISC = 0


def _build_program(debug=False, loop_n=None):
    nc = bacc.Bacc()
    xs = nc.declare_dram_parameter("xs", [C, SQ], F32, isOutput=False)
    ys = nc.declare_dram_parameter("ys", [C, SK], F32, isOutput=False)
    cpack = nc.declare_dram_parameter("cpack", [128, 779], F32, isOutput=False)
    cpackb = nc.declare_dram_parameter("cpackb", [128, 256], BF16, isOutput=False)
    cpackr = nc.declare_dram_parameter("cpackr", [128, 384], F32R, isOutput=False)
    outp = nc.declare_dram_parameter("outp", [SQ, 128], F32, isOutput=True)
    outp_r = outp.rearrange("(n k p) c -> n p k c", k=SC // 128, p=128)
    if debug:
        dbg_t = nc.declare_dram_parameter("dbg_t", [128, SC], F32, isOutput=True)
        dbg_kf = nc.declare_dram_parameter("dbg_kf", [128, SC], F32, isOutput=True)
        dbg_v = nc.declare_dram_parameter("dbg_v", [128, SC], F32, isOutput=True)
        dbg_kvb = nc.declare_dram_parameter("dbg_kvb", [128, 136], F32, isOutput=True)
        dbg_qs = nc.declare_dram_parameter("dbg_qs", [128, SC], F32, isOutput=True)
        dbg_nm = nc.declare_dram_parameter("dbg_nm", [128, SC // 128, 8, 17], F32,
                                           isOutput=True)

    with tile.TileContext(nc) as tc, ExitStack() as ctx:
        singles = ctx.enter_context(tc.tile_pool(name="singles", bufs=1))
        inpool = ctx.enter_context(tc.tile_pool(name="inpool", bufs=3))
        tpool = ctx.enter_context(tc.tile_pool(name="tpool", bufs=2))
        fpool = ctx.enter_context(tc.tile_pool(name="fpool", bufs=2))
        vpool = ctx.enter_context(tc.tile_pool(name="vpool", bufs=2))
        npool = ctx.enter_context(tc.tile_pool(name="npool", bufs=2))
        opool = ctx.enter_context(tc.tile_pool(name="opool", bufs=2))
        pps = ctx.enter_context(tc.tile_pool(name="pps", bufs=2, space="PSUM"))
        ppkv = ctx.enter_context(tc.tile_pool(name="ppkv", bufs=1, space="PSUM"))

        consts = singles.tile([128, 779], F32)
        nc.sync.dma_start(out=consts, in_=cpack[:])
        wqt = consts[:, 0:128]
        wkt = consts[:, 128:256]
        bmat = consts[:, 256:384]
        fmat = consts[:, 384:512]
        ident = consts[:, 512:640]
        bq = consts[:, 640:641]
        bk = consts[:, 641:642]
        mask136 = consts[:, 642:778]
        ones1 = consts[:, 778:779]
        constsb = singles.tile([128, 256], BF16)
        nc.sync.dma_start(out=constsb, in_=cpackb[:])
        bmat_bf = constsb[:, 0:128]
        fmat_bf = constsb[:, 128:256]
        constsr = singles.tile([128, 384], F32R)
        nc.sync.dma_start(out=constsr, in_=cpackr[:])
        wqt_r = constsr[:, 0:128]
        wkt_r = constsr[:, 128:256]
        ident_r = constsr[:, 256:384]

        # Tiny ops so PE/DVE observe the consts DMA once up front; later
        # instructions then carry at most one new semaphore wait each
        # (walrus per-instruction sync-command slot limit).
        presync = pps.tile([128, 512], F32, tag="tps")
        nc.tensor.matmul(presync[:1, 0:1], lhsT=consts[:, 0:1],
                         rhs=consts[:, 0:1], start=True, stop=True)
        dve_sync = singles.tile([128, 1], F32)
        nc.vector.tensor_copy(dve_sync, consts[:, 0:1])

        def body():
            # ---- K phase: KV[hm, hv] (+ ksum col) accumulated in PSUM ----
            kvps = ppkv.tile([128, 129], F32, tag="kv")
            nkc = SK // SC
            for isc in range(nkc):
                y_t = inpool.tile([128, SC], F32R, tag="inbuf")
                nc.sync.dma_start(out=y_t, in_=ys[:, isc * SC:(isc + 1) * SC].bitcast(F32R))
                t_t = tpool.tile([128, SC], BF16, tag="t")
                t2_t = tpool.tile([128, SC], BF16, tag="t2")
                for j in range(SC // 512):
                    blk = slice(j * 512, (j + 1) * 512)
                    tps = pps.tile([128, 512], F32, tag="tps")
                    nc.tensor.matmul(tps, lhsT=wkt_r, rhs=y_t[:, blk],
                                     start=True, stop=True)
                    nc.scalar.activation(t_t[:, blk], tps, AF.Identity, bias=bk)
                    nc.vector.tensor_mul(t2_t[:, blk], t_t[:, blk], t_t[:, blk])
                kf_t = fpool.tile([128, SC], BF16, tag="feat")
                v_t = vpool.tile([128, SC // 128, 129], BF16, tag="v")
                nc.vector.memset(v_t[:, :, 128:129], 1.0)
                for j in range(SC // 512):
                    blk = slice(j * 512, (j + 1) * 512)
                    kfps = pps.tile([128, 512], F32, tag="kfps")
                    vps = pps.tile([128, 512], F32R, tag="vps")
                    for c in range(4):
                        cc = j * 4 + c
                        sl = slice(c * 128, (c + 1) * 128)
                        ch = slice(cc * 128, (cc + 1) * 128)
                        nc.tensor.matmul(kfps[:, sl], lhsT=t_t[:, ch], rhs=bmat_bf,
                                         start=True, stop=False)
                        nc.tensor.matmul(kfps[:, sl], lhsT=t2_t[:, ch], rhs=fmat_bf,
                                         start=False, stop=True)
                        nc.tensor.transpose(vps[:, sl], y_t[:, ch], ident_r)
                    nc.scalar.activation(kf_t[:, blk], kfps, AF.Exp)
                    nc.vector.tensor_copy(
                        v_t[:, j * 4:(j + 1) * 4, 0:128],
                        vps.rearrange("p (c f) -> p c f", c=4))

                for cc in range(SC // 128):
                    ch = slice(cc * 128, (cc + 1) * 128)
                    gfirst = (isc == 0) and (cc == 0)
                    last = (isc == nkc - 1) and (cc == SC // 128 - 1)
                    nc.tensor.matmul(kvps, lhsT=kf_t[:, ch],
                                     rhs=v_t[:, cc, :], start=gfirst, stop=last,
                                     skip_group_check=True)
                if debug and isc == DBG_KISC:
                    nc.sync.dma_start(out=dbg_t[:], in_=t_t)
                    nc.sync.dma_start(out=dbg_kf[:], in_=kf_t)
                    nc.sync.dma_start(out=dbg_v[:], in_=v_t)

            # ---- KVB: block-diagonal [KV | ksum] for the num matmul ----
            kvsb = npool.tile([128, 129], F32, tag="kvsb")
            nc.vector.tensor_copy(kvsb, kvps)
            kvb_raw = npool.tile([128, 8, 17], F32, tag="kvbr")
            nc.vector.tensor_copy(kvb_raw[:, :, 0:16],
                                  kvsb[:, 0:128].rearrange("p (h j) -> p h j", h=8))
            nc.vector.tensor_copy(kvb_raw[:, :, 16:17],
                                  kvsb[:, 128:129, None].to_broadcast([128, 8, 1]))
            kvb3 = npool.tile([128, 8, 17], F32, tag="kvb3")
            nc.vector.tensor_mul(kvb3, kvb_raw,
                                 mask136.rearrange("p (h j) -> p h j", h=8))
            kvb = kvb3.rearrange("p h j -> p (h j)")
            if debug:
                nc.sync.dma_start(out=dbg_kvb[:], in_=kvb)

            # ---- Q phase ----
            for isc in range(SQ // SC):
                x_t = inpool.tile([128, SC], F32R, tag="inbuf")
                nc.sync.dma_start(out=x_t, in_=xs[:, isc * SC:(isc + 1) * SC].bitcast(F32R))
                t_t = tpool.tile([128, SC], BF16, tag="t")
                t2_t = tpool.tile([128, SC], BF16, tag="t2")
                for j in range(SC // 512):
                    blk = slice(j * 512, (j + 1) * 512)
                    tps = pps.tile([128, 512], F32, tag="tps")
                    nc.tensor.matmul(tps, lhsT=wqt_r, rhs=x_t[:, blk],
                                     start=True, stop=True)
                    nc.scalar.activation(t_t[:, blk], tps, AF.Identity, bias=bq)
                    nc.vector.tensor_mul(t2_t[:, blk], t_t[:, blk], t_t[:, blk])
                qs_t = fpool.tile([128, SC], F32, tag="feat")
                for j in range(SC // 512):
                    blk = slice(j * 512, (j + 1) * 512)
                    qps = pps.tile([128, 512], F32, tag="kfps")
                    nc.tensor.matmul(qps, lhsT=bmat_bf, rhs=t_t[:, blk],
                                     start=True, stop=False)
                    nc.tensor.matmul(qps, lhsT=fmat_bf, rhs=t2_t[:, blk],
                                     start=False, stop=True)
                    nc.scalar.activation(qs_t[:, blk], qps, AF.Exp)
                nm_t = npool.tile([128, SC // 128, 8, 17], F32, tag="nm")
                for cc in range(SC // 128):
                    ch = slice(cc * 128, (cc + 1) * 128)
                    nps = pps.tile([128, 136], F32, tag="vps")
                    nc.tensor.matmul(nps, lhsT=qs_t[:, ch], rhs=kvb,
                                     start=True, stop=True)
                    nc.vector.tensor_copy(nm_t[:, cc],
                                          nps.rearrange("p (h j) -> p h j", h=8))
                if debug and isc == 0:
                    nc.sync.dma_start(out=dbg_qs[:], in_=qs_t)
                    nc.sync.dma_start(out=dbg_nm[:], in_=nm_t)
                rden = npool.tile([128, SC // 128, 8], F32, tag="rden")
                nc.vector.reciprocal(rden, nm_t[:, :, :, 16])
                out_t = opool.tile([128, SC // 128, 128], F32, tag="outb")
                for h in range(N_HEADS):
                    nc.vector.tensor_mul(
                        out_t[:, :, 16 * h:16 * h + 16],
                        nm_t[:, :, h, 0:16],
                        rden[:, :, h, None].to_broadcast([128, SC // 128, 16]),
                    )
                nc.sync.dma_start(out=outp_r[isc], in_=out_t)

        if loop_n is None:
            body()
        else:
            with tc.For_i(0, loop_n, 1):
                body()

    nc.compile()
    return nc


def _get_program():
    global _PROGRAM
    if _PROGRAM is None:
        _PROGRAM = _build_program()
    return _PROGRAM


def _host_consts(rfs, Wq, bq, Wk, bk):
    scale = HD ** -0.25  # == 0.5 exactly
    cpack = np.zeros((128, 779), dtype=np.float32)
    cpack[:, 0:128] = (scale * Wq).T
    cpack[:, 128:256] = (scale * Wk).T
    bmat = np.zeros((128, 128), dtype=np.float32)
    fmat = np.zeros((128, 128), dtype=np.float32)
    for h in range(N_HEADS):
        bmat[16 * h:16 * h + 16, 16 * h:16 * h + 16] = rfs[h]
        fmat[16 * h:16 * h + 16, 16 * h:16 * h + 16] = -0.5
    cpack[:, 256:384] = bmat
    cpack[:, 384:512] = fmat
    cpack[:, 512:640] = np.eye(128, dtype=np.float32)
    cpack[:, 640] = scale * bq
    cpack[:, 641] = scale * bk
    mask = np.zeros((128, 136), dtype=np.float32)
    for h in range(N_HEADS):
        mask[16 * h:16 * h + 16, 17 * h:17 * h + 17] = 1.0
    cpack[:, 642:778] = mask
    cpack[:, 778] = 1.0
    import ml_dtypes
    cpackb = np.concatenate([bmat, fmat], axis=1).astype(ml_dtypes.bfloat16)
    cpackr = np.ascontiguousarray(np.concatenate(
        [cpack[:, 0:256], np.eye(128, dtype=np.float32)], axis=1))
    return cpack, cpackb, cpackr


def make_in_maps(inputs):
    x = np.ascontiguousarray(np.asarray(inputs["x"], dtype=np.float32))
    y = np.ascontiguousarray(np.asarray(inputs["y"], dtype=np.float32))
    cpack, cpackb, cpackr = _host_consts(np.asarray(inputs["rfs"], dtype=np.float32),
                         np.asarray(inputs["Wq"], dtype=np.float32),
                         np.asarray(inputs["bq"], dtype=np.float32),
                         np.asarray(inputs["Wk"], dtype=np.float32),
                         np.asarray(inputs["bk"], dtype=np.float32))
    B = x.shape[0]
    xr = x.reshape(B, C, S)
    yr = y.reshape(B, C, S)
    in_maps = []
    for core in range(8):
        b, half = core // 2, core % 2
        s0 = half * SQ
        in_maps.append({
            "xs": np.ascontiguousarray(xr[b][:, s0:s0 + SQ]),
            "ys": np.ascontiguosarray(yr[b]) if False else np.ascontiguousarray(yr[b]),
            "cpack": cpack,
            "cpackb": cpackb,
            "cpackr": cpackr,
        })
    return in_maps


def run(inputs, trace=False, **kwargs):
    in_maps = make_in_maps(inputs)
    nc = _get_program()
    res = run_bass_kernel_spmd(nc, in_maps, list(range(8)), trace=trace, **kwargs)
    B = np.asarray(inputs["x"]).shape[0]
    out = np.empty((B, S, 128), dtype=np.float32)
    for core in range(8):
        b, half = core // 2, core % 2
        s0 = half * SQ
        out[b, s0:s0 + SQ, :] = res.results[core]["outp"]
    return out.reshape(np.asarray(inputs["x"]).shape), res


def kernel(**inputs):
    out, _ = run(inputs, trace=False)
    return out


# revision 40
# speedup vs baseline: 2.7969x; 1.0411x over previous
"""FAVOR+ linear attention (Performer-style) Trainium2 Bass kernel.

Full inputs -> full output. Sharding: 8 cores = (batch b in 0..3) x (query
half in 0..1). Each core computes the KV summary over all 16384 key pixels
of its batch (duplicated within the pair; avoids collectives) and the
numerator/denominator for its 8192 query pixels.

Hardcoded problem shape: B=4, C=128, H=W=128, hid=128, heads=8, hd=dv=16.
"""

import numpy as np
from contextlib import ExitStack

import concourse.bass as bass
import concourse.tile as tile
from concourse import bacc, mybir
from concourse.bass_utils import run_bass_kernel_spmd

F32 = mybir.dt.float32
F32R = mybir.dt.float32r
BF16 = mybir.dt.bfloat16
AF = mybir.ActivationFunctionType


def _r(ap):
    return ap.bitcast(F32R)

N_HEADS = 8
HD = 16          # head dim for q/k and v
C = 128          # channels == hid
S = 128 * 128    # pixels per image
SQ = S // 2      # query pixels per core
SK = S           # key pixels per core (duplicated across the pair)
SC = 2048        # super-chunk (pixels)

_PROGRAM = None
DBG_KISC = 0


def _build_program(debug=False, loop_n=None, seqpar=False):
    nc = bacc.Bacc()
    sk = SQ if seqpar else SK
    xs = nc.declare_dram_parameter("xs", [C, SQ], F32, isOutput=False)
    ys = nc.declare_dram_parameter("ys", [C, sk], F32, isOutput=False)
    if seqpar:
        kv_in = nc.dram_tensor("kv_in", [128, 129], F32)
        kv_sh = nc.dram_tensor("kv_sh", [128, 129], F32)
    cpack = nc.declare_dram_parameter("cpack", [128, 779], F32, isOutput=False)
    cpackb = nc.declare_dram_parameter("cpackb", [128, 256], BF16, isOutput=False)
    cpackr = nc.declare_dram_parameter("cpackr", [128, 384], F32R, isOutput=False)
    outp = nc.declare_dram_parameter("outp", [SQ, 128], F32, isOutput=True)
    outp_r = outp.rearrange("(n k p) c -> n p k c", k=SC // 128, p=128)
    if debug:
        dbg_t = nc.declare_dram_parameter("dbg_t", [128, SC], F32, isOutput=True)
        dbg_kf = nc.declare_dram_parameter("dbg_kf", [128, SC], F32, isOutput=True)
        dbg_v = nc.declare_dram_parameter("dbg_v", [128, SC], F32, isOutput=True)
        dbg_kvb = nc.declare_dram_parameter("dbg_kvb", [128, 136], F32, isOutput=True)
        dbg_qs = nc.declare_dram_parameter("dbg_qs", [128, SC], F32, isOutput=True)
        dbg_nm = nc.declare_dram_parameter("dbg_nm", [128, SC // 128, 8, 17], F32,
                                           isOutput=True)

    with tile.TileContext(nc) as tc, ExitStack() as ctx:
        singles = ctx.enter_context(tc.tile_pool(name="singles", bufs=1))
        inpool = ctx.enter_context(tc.tile_pool(name="inpool", bufs=3))
        tpool = ctx.enter_context(tc.tile_pool(name="tpool", bufs=2))
        fpool = ctx.enter_context(tc.tile_pool(name="fpool", bufs=2))
        vpool = ctx.enter_context(tc.tile_pool(name="vpool", bufs=2))
        npool = ctx.enter_context(tc.tile_pool(name="npool", bufs=2))
        opool = ctx.enter_context(tc.tile_pool(name="opool", bufs=2))
        pps = ctx.enter_context(tc.tile_pool(name="pps", bufs=2, space="PSUM"))
        ppkv = ctx.enter_context(tc.tile_pool(name="ppkv", bufs=1, space="PSUM"))

        consts = singles.tile([128, 779], F32)
        nc.sync.dma_start(out=consts, in_=cpack[:])
        wqt = consts[:, 0:128]
        wkt = consts[:, 128:256]
        bmat = consts[:, 256:384]
        fmat = consts[:, 384:512]
        ident = consts[:, 512:640]
        bq = consts[:, 640:641]
        bk = consts[:, 641:642]
        mask136 = consts[:, 642:778]
        ones1 = consts[:, 778:779]
        constsb = singles.tile([128, 256], BF16)
        nc.sync.dma_start(out=constsb, in_=cpackb[:])
        bmat_bf = constsb[:, 0:128]
        fmat_bf = constsb[:, 128:256]
        constsr = singles.tile([128, 384], F32R)
        nc.sync.dma_start(out=constsr, in_=cpackr[:])
        wqt_r = constsr[:, 0:128]
        wkt_r = constsr[:, 128:256]
        ident_r = constsr[:, 256:384]

        # Tiny ops so PE/DVE observe the consts DMA once up front; later
        # instructions then carry at most one new semaphore wait each
        # (walrus per-instruction sync-command slot limit).
        presync = pps.tile([128, 512], F32, tag="tps")
        nc.tensor.matmul(presync[:1, 0:1], lhsT=consts[:, 0:1],
                         rhs=consts[:, 0:1], start=True, stop=True)
        dve_sync = singles.tile([128, 1], F32)
        nc.vector.tensor_copy(dve_sync, consts[:, 0:1])

        def body():
            # ---- K phase: KV[hm, hv] (+ ksum col) accumulated in PSUM ----
            kvps = ppkv.tile([128, 129], F32, tag="kv")
            nkc = sk // SC
            for isc in range(nkc):
                y_t = inpool.tile([128, SC], F32R, tag="inbuf")
                nc.sync.dma_start(out=y_t, in_=ys[:, isc * SC:(isc + 1) * SC].bitcast(F32R))
                t_t = tpool.tile([128, SC], BF16, tag="t")
                t2_t = tpool.tile([128, SC], BF16, tag="t2")
                for j in range(SC // 512):
                    blk = slice(j * 512, (j + 1) * 512)
                    tps = pps.tile([128, 512], F32, tag="tps")
                    nc.tensor.matmul(tps, lhsT=wkt_r, rhs=y_t[:, blk],
                                     start=True, stop=True)
                    nc.scalar.activation(t_t[:, blk], tps, AF.Identity, bias=bk)
                    nc.vector.tensor_mul(t2_t[:, blk], t_t[:, blk], t_t[:, blk])
                kf_t = fpool.tile([128, SC], BF16, tag="feat")
                v_t = vpool.tile([128, SC // 128, 129], BF16, tag="v")
                nc.vector.memset(v_t[:, :, 128:129], 1.0)
                for j in range(SC // 512):
                    blk = slice(j * 512, (j + 1) * 512)
                    kfps = pps.tile([128, 512], F32, tag="kfps")
                    vps = pps.tile([128, 512], F32R, tag="vps")
                    for c in range(4):
                        cc = j * 4 + c
                        sl = slice(c * 128, (c + 1) * 128)
                        ch = slice(cc * 128, (cc + 1) * 128)
                        nc.tensor.matmul(kfps[:, sl], lhsT=t_t[:, ch], rhs=bmat_bf,
                                         start=True, stop=False)
                        nc.tensor.matmul(kfps[:, sl], lhsT=t2_t[:, ch], rhs=fmat_bf,
                                         start=False, stop=True)
                        nc.tensor.transpose(vps[:, sl], y_t[:, ch], ident_r)
                    nc.scalar.activation(kf_t[:, blk], kfps, AF.Exp)
                    nc.vector.tensor_copy(
                        v_t[:, j * 4:(j + 1) * 4, 0:128],
                        vps.rearrange("p (c f) -> p c f", c=4))

                for cc in range(SC // 128):
                    ch = slice(cc * 128, (cc + 1) * 128)
                    gfirst = (isc == 0) and (cc == 0)
                    last = (isc == nkc - 1) and (cc == SC // 128 - 1)
                    nc.tensor.matmul(kvps, lhsT=kf_t[:, ch],
                                     rhs=v_t[:, cc, :], start=gfirst, stop=last,
                                     skip_group_check=True)
                if debug and isc == DBG_KISC:
                    nc.sync.dma_start(out=dbg_t[:], in_=t_t)
                    nc.sync.dma_start(out=dbg_kf[:], in_=kf_t)
                    nc.sync.dma_start(out=dbg_v[:], in_=v_t)

            # ---- KVB: block-diagonal [KV | ksum] for the num matmul ----
            kvsb = npool.tile([128, 129], F32, tag="kvsb")
            nc.vector.tensor_copy(kvsb, kvps)
            if seqpar:
                nc.sync.dma_start(out=kv_in[:], in_=kvsb)
                nc.gpsimd.collective_compute(
                    "AllReduce", mybir.AluOpType.add,
                    ins=[kv_in[:]], outs=[kv_sh[:]],
                    replica_groups=[[0, 1], [2, 3], [4, 5], [6, 7]])
                kvsb = npool.tile([128, 129], F32, tag="kvsb2")
                nc.sync.dma_start(out=kvsb, in_=kv_sh[:])
            kvb_raw = npool.tile([128, 8, 17], F32, tag="kvbr")
            nc.vector.tensor_copy(kvb_raw[:, :, 0:16],
                                  kvsb[:, 0:128].rearrange("p (h j) -> p h j", h=8))
            nc.vector.tensor_copy(kvb_raw[:, :, 16:17],
                                  kvsb[:, 128:129, None].to_broadcast([128, 8, 1]))
            kvb3 = npool.tile([128, 8, 17], BF16, tag="kvb3")
            nc.vector.tensor_mul(kvb3, kvb_raw,
                                 mask136.rearrange("p (h j) -> p h j", h=8))
            kvb = kvb3.rearrange("p h j -> p (h j)")
            if debug:
                nc.sync.dma_start(out=dbg_kvb[:], in_=kvb)

            # ---- Q phase ----
            for isc in range(SQ // SC):
                x_t = inpool.tile([128, SC], F32R, tag="inbuf")
                nc.sync.dma_start(out=x_t, in_=xs[:, isc * SC:(isc + 1) * SC].bitcast(F32R))
                t_t = tpool.tile([128, SC], BF16, tag="t")
                t2_t = tpool.tile([128, SC], BF16, tag="t2")
                for j in range(SC // 512):
                    blk = slice(j * 512, (j + 1) * 512)
                    tps = pps.tile([128, 512], F32, tag="tps")
                    nc.tensor.matmul(tps, lhsT=wqt_r, rhs=x_t[:, blk],
                                     start=True, stop=True)
                    nc.scalar.activation(t_t[:, blk], tps, AF.Identity, bias=bq)
                    nc.vector.tensor_mul(t2_t[:, blk], t_t[:, blk], t_t[:, blk])
                qs_t = fpool.tile([128, SC], BF16, tag="feat")
                for j in range(SC // 512):
                    blk = slice(j * 512, (j + 1) * 512)
                    qps = pps.tile([128, 512], F32, tag="kfps")
                    nc.tensor.matmul(qps, lhsT=bmat_bf, rhs=t_t[:, blk],
                                     start=True, stop=False)
                    nc.tensor.matmul(qps, lhsT=fmat_bf, rhs=t2_t[:, blk],
                                     start=False, stop=True)
                    nc.scalar.activation(qs_t[:, blk], qps, AF.Exp)
                nm_t = npool.tile([128, SC // 128, 8, 17], F32, tag="nm")
                cc = 0
                while cc < SC // 128:
                    g = min(3, SC // 128 - cc)
                    nps = pps.tile([128, 3, 136], F32, tag="vps")
                    for i in range(g):
                        ch = slice((cc + i) * 128, (cc + i + 1) * 128)
                        nc.tensor.matmul(nps[:, i, :], lhsT=qs_t[:, ch], rhs=kvb,
                                         start=True, stop=True)
                    nc.vector.tensor_copy(
                        nm_t[:, cc:cc + g],
                        nps[:, 0:g, :].rearrange("p c (h j) -> p c h j", h=8))
                    cc += g
                if debug and isc == 0:
                    nc.sync.dma_start(out=dbg_qs[:], in_=qs_t)
                    nc.sync.dma_start(out=dbg_nm[:], in_=nm_t)
                rden = npool.tile([128, SC // 128, 8], F32, tag="rden")
                nc.vector.reciprocal(rden, nm_t[:, :, :, 16])
                out_t = opool.tile([128, SC // 128, 128], F32, tag="outb")
                for h in range(N_HEADS):
                    nc.vector.tensor_mul(
                        out_t[:, :, 16 * h:16 * h + 16],
                        nm_t[:, :, h, 0:16],
                        rden[:, :, h, None].to_broadcast([128, SC // 128, 16]),
                    )
                nc.sync.dma_start(out=outp_r[isc], in_=out_t)

        if loop_n is None:
            body()
        else:
            with tc.For_i(0, loop_n, 1):
                body()

    nc.compile()
    return nc


def _get_program():
    global _PROGRAM
    if _PROGRAM is None:
        _PROGRAM = _build_program(seqpar=SEQPAR)
    return _PROGRAM


def _host_consts(rfs, Wq, bq, Wk, bk):
    scale = HD ** -0.25  # == 0.5 exactly
    cpack = np.zeros((128, 779), dtype=np.float32)
    cpack[:, 0:128] = (scale * Wq).T
    cpack[:, 128:256] = (scale * Wk).T
    bmat = np.zeros((128, 128), dtype=np.float32)
    fmat = np.zeros((128, 128), dtype=np.float32)
    for h in range(N_HEADS):
        bmat[16 * h:16 * h + 16, 16 * h:16 * h + 16] = rfs[h]
        fmat[16 * h:16 * h + 16, 16 * h:16 * h + 16] = -0.5
    cpack[:, 256:384] = bmat
    cpack[:, 384:512] = fmat
    cpack[:, 512:640] = np.eye(128, dtype=np.float32)
    cpack[:, 640] = scale * bq
    cpack[:, 641] = scale * bk
    mask = np.zeros((128, 136), dtype=np.float32)
    for h in range(N_HEADS):
        mask[16 * h:16 * h + 16, 17 * h:17 * h + 17] = 1.0
    cpack[:, 642:778] = mask
    cpack[:, 778] = 1.0
    import ml_dtypes
    cpackb = np.concatenate([bmat, fmat], axis=1).astype(ml_dtypes.bfloat16)
    cpackr = np.ascontiguousarray(np.concatenate(
        [cpack[:, 0:256], np.eye(128, dtype=np.float32)], axis=1))
    return cpack, cpackb, cpackr


SEQPAR = False


def make_in_maps(inputs):
    x = np.ascontiguousarray(np.asarray(inputs["x"], dtype=np.float32))
    y = np.ascontiguousarray(np.asarray(inputs["y"], dtype=np.float32))
    cpack, cpackb, cpackr = _host_consts(np.asarray(inputs["rfs"], dtype=np.float32),
                         np.asarray(inputs["Wq"], dtype=np.float32),
                         np.asarray(inputs["bq"], dtype=np.float32),
                         np.asarray(inputs["Wk"], dtype=np.float32),
                         np.asarray(inputs["bk"], dtype=np.float32))
    B = x.shape[0]
    xr = x.reshape(B, C, S)
    yr = y.reshape(B, C, S)
    in_maps = []
    for core in range(8):
        b, half = core // 2, core % 2
        s0 = half * SQ
        ys_i = yr[b][:, s0:s0 + SQ] if SEQPAR else yr[b]
        in_maps.append({
            "xs": np.ascontiguousarray(xr[b][:, s0:s0 + SQ]),
            "ys": np.ascontiguousarray(ys_i),
            "cpack": cpack,
            "cpackb": cpackb,
            "cpackr": cpackr,
        })
    return in_maps


def run(inputs, trace=False, **kwargs):
    in_maps = make_in_maps(inputs)
    nc = _get_program()
    res = run_bass_kernel_spmd(nc, in_maps, list(range(8)), trace=trace, **kwargs)
    B = np.asarray(inputs["x"]).shape[0]
    out = np.empty((B, S, 128), dtype=np.float32)
    for core in range(8):
        b, half = core // 2, core % 2
        s0 = half * SQ
        out[b, s0:s0 + SQ, :] = res.results[core]["outp"]
    return out.reshape(np.asarray(inputs["x"]).shape), res


def kernel(**inputs):
    out, _ = run(inputs, trace=False)
    return out
